# revision 21
# baseline (speedup 1.0000x reference)
"""CRF forward-backward marginals on 8 Trainium2 NeuronCores.

Strategy (hardcoded for B=64, T=512, D=1024, K=32, 8 cores):
  - Data-parallel over batch: core i handles batches [8i, 8i+8).
  - Host-side prep: x^T bf16 per core (no on-chip transposes, half DMA);
    constants precomputed on host: wn = W - W[:,0] (bf16, chunk-major),
    bn = b - b[0] + log(1/(K*e)), blk = diag(exp(U), exp(U)^T),
    idk33 = [I_32 | ones] for fused transpose+rowsum.
  - Emissions: E'^T = exp(x @ wn + bn) via bf16 matmul, d-chunk-outer with
    8 concurrent PSUM groups so DMA overlaps the accumulating matmuls.
  - Fused scan: states for both directions live in one [64, ...] tile
    (fwd partitions 0:32, bwd 32:64, both step-indexed); one [64,64]
    block-diagonal matmul + one tensor_tensor per step:
      fwd:  p_s = (p_{s-1} @ eUn) * Em_f[s]
      bwd:  w_s = (w_{s-1} @ eUn^T) * Em_b[s]   (Em_b position-reversed)
    Time-parallelized over 32 chunks of 16 steps with 8 burn-in steps;
    chunk 0 fwd / chunk 31 bwd exactly re-init after burn-in.
    V (fwd pre-multiply) copied per step by Scalar; bwd states mirrored to
    a combine-ready base-0 tile by GpSimd.
  - Combine + PE transpose (with ones-column producing rowsums for free) +
    rownorm + 512B-contiguous DMA out.
"""

import os
import sys

import numpy as np
import ml_dtypes

sys.path.insert(0, "/opt/trn_rl_repo")

import concourse.bass as bass  # noqa: E402
import concourse.bacc as bacc  # noqa: E402
import concourse.mybir as mybir  # noqa: E402
from concourse import tile  # noqa: E402

B, T, D, K = 64, 512, 1024, 32
NCORES = 8
BL = B // NCORES            # 8 batches per core
ROWS = BL * T               # 4096 rows per core
S_CH = 16                   # chunk length
V_BI = 4                    # burn-in positions
C_CH = T // S_CH            # 32 chunks
CHAINS = BL * C_CH          # 256 parallel chains
POS = S_CH + V_BI           # 24 scan positions per direction
LOG_CU = -(np.log(K) + 1.0)  # log(1/(K*e)) folded into exp(U)

f32 = mybir.dt.float32
f32r = mybir.dt.float32r
bf16 = mybir.dt.bfloat16
AX = mybir.AxisListType
ALU = mybir.AluOpType
ACTF = mybir.ActivationFunctionType


def build_nc(finalize=True):
    nc = bacc.Bacc("TRN2", target_bir_lowering=False)
    x_h = nc.declare_dram_parameter("x", [D, ROWS], bf16, isOutput=False)
    wn_h = nc.declare_dram_parameter("wn", [128, 8 * K], bf16, isOutput=False)
    blk_h = nc.declare_dram_parameter("blk", [2 * K, 2 * K], bf16, isOutput=False)
    bn_h = nc.declare_dram_parameter("bn", [K, 1], f32, isOutput=False)
    idk_h = nc.declare_dram_parameter("idk", [K, K + 1], bf16, isOutput=False)
    o_h = nc.declare_dram_parameter("out", [ROWS, K], f32, isOutput=True)

    with tile.TileContext(nc) as tc:
        with (
            tc.tile_pool(name="const", bufs=1) as cpool,
            tc.tile_pool(name="stores", bufs=1) as spool,
        ):
            # constants on the scalar queue (tiny), x on the sync queue.
            wn3 = cpool.tile([128, 8, K], bf16)
            nc.scalar.dma_start(wn3[:], wn_h.ap().rearrange("p (n k) -> p n k", n=8))
            blk = cpool.tile([2 * K, 2 * K], bf16)
            nc.scalar.dma_start(blk[:], blk_h.ap())
            bn = cpool.tile([K, 1], f32)
            nc.scalar.dma_start(bn[:], bn_h.ap())
            idk = cpool.tile([K, K + 1], bf16)
            nc.scalar.dma_start(idk[:], idk_h.ap())

            # x: first d-chunk in 4 column pieces so the first matmuls can
            # start early; remaining chunks as whole 1MB tiles.
            xt = spool.tile([128, 8, ROWS], bf16)
            for p in range(4):
                nc.sync.dma_start(
                    xt[:, 0, p * 1024:(p + 1) * 1024],
                    x_h.ap()[0:128, p * 1024:(p + 1) * 1024])
            for db in (1, 2):
                for hh in range(2):
                    nc.sync.dma_start(
                        xt[:, db, hh * 2048:(hh + 1) * 2048],
                        x_h.ap()[db * 128:(db + 1) * 128,
                                 hh * 2048:(hh + 1) * 2048])
            for db in range(3, 8):
                nc.sync.dma_start(xt[:, db, :],
                                  x_h.ap()[db * 128:(db + 1) * 128, :])

            # EM[p, b, s, c]: step-indexed emissions; fwd half (p<32) row s =
            # E'[16c + s - 8], bwd half row s = E'[16c + 23 - s]
            CU = float(np.exp(LOG_CU))
            EM = spool.tile([2 * K, BL, POS, C_CH], f32)
            S2 = spool.tile([2 * K, BL, POS, C_CH], bf16)
            Sb0h = spool.tile([2 * K, BL, S_CH, C_CH], bf16)
            nc.gpsimd.memset(EM[0:K, :, 0:V_BI, 0], CU)
            nc.gpsimd.memset(EM[K:2 * K, :, 0:V_BI, C_CH - 1], CU)

            warm = cpool.tile([128, 512], bf16)
            nc.gpsimd.memset(warm[:], 0.0)

            # ------------- emission -------------
            with tc.tile_pool(name="ps_e", bufs=1, space="PSUM") as ps_e_pool:
                e_ps = [ps_e_pool.tile([K, 512], f32, tag=f"e{st}", name=f"e_ps{st}")
                        for st in range(BL)]
                # dummy matmuls ramp the PE p-state while x streams in;
                # results land in e_ps[0] and are overwritten by the real
                # start=True matmul
                for w in range(4):
                    nc.tensor.matmul(e_ps[0][:], warm[:, 0:K], warm[:],
                                     start=True, stop=True)
                for db in range(8):
                    for st in range(BL):
                        nc.tensor.matmul(
                            e_ps[st][:], wn3[:, db, :],
                            xt[:, db, st * 512:(st + 1) * 512],
                            start=(db == 0), stop=(db == 7),
                        )
                        if db == 7:
                            eview = e_ps[st][:].rearrange("k (c u) -> k u c",
                                                          u=S_CH)
                            # fwd main rows [V_BI, V_BI+16)
                            nc.scalar.activation(
                                EM[0:K, st, V_BI:V_BI + 16, :], eview,
                                ACTF.Exp, bias=bn[:, 0:1])
                            # bwd main rows: row (V_BI+15-u) = fwd main row
                            # V_BI+u (position-reversed copy) on Scalar so
                            # Vector is free for the early scan steps
                            nc.scalar.activation(
                                EM[K:2 * K, st, :, :][
                                    :, V_BI + 15:V_BI - 1:-1, :],
                                EM[0:K, st, V_BI:V_BI + 16, :], ACTF.Copy)
                            # head dups (rows 0..V_BI): fwd from chunk c-1,
                            # bwd from chunk c+1 (+16 rule)
                            nc.vector.tensor_copy(
                                EM[0:K, st, 0:V_BI, 1:C_CH],
                                EM[0:K, st, 16:16 + V_BI, 0:C_CH - 1])
                            nc.vector.tensor_copy(
                                EM[K:2 * K, st, 0:V_BI, 0:C_CH - 1],
                                EM[K:2 * K, st, 16:16 + V_BI, 1:C_CH])

            # ------------- fused scan + combine + output -------------
            with (
                tc.tile_pool(name="outsb", bufs=3) as opool,
                tc.tile_pool(name="ps_s", bufs=2, space="PSUM") as ps_s_pool,
                tc.tile_pool(name="ps_v", bufs=2, space="PSUM") as ps_v_pool,
                tc.tile_pool(name="ps_o", bufs=2, space="PSUM") as ps_o_pool,
            ):
                blk_r = blk[:]
                HB = BL // 2
                for s in range(POS):
                    if s == 0:
                        nc.vector.tensor_copy(S2[:, 0:HB, 0, :],
                                              EM[:, 0:HB, 0, :])
                        nc.vector.tensor_copy(S2[:, HB:BL, 0, :],
                                              EM[:, HB:BL, 0, :])
                        continue
                    # two independent half-batch chains so the MM of one half
                    # overlaps the TT of the other (hides serial latency)
                    for h in range(2):
                        bs = slice(h * HB, (h + 1) * HB)
                        ps2 = ps_s_pool.tile([2 * K, HB * C_CH], f32,
                                             tag=f"ps{h}")
                        nc.tensor.matmul(ps2[:], blk_r,
                                         S2[:, bs, s - 1, :].opt(),
                                         start=True, stop=True)
                        ps3 = ps2[:].rearrange("p (b c) -> p b c", b=HB)
                        nc.vector.tensor_tensor(S2[:, bs, s, :], ps3,
                                                EM[:, bs, s, :], op=ALU.mult)
                    if s == V_BI:
                        # exact re-inits once burn-in is done
                        nc.vector.tensor_copy(S2[0:K, :, V_BI, 0],
                                              EM[0:K, :, V_BI, 0])
                        nc.vector.tensor_copy(S2[K:2 * K, :, V_BI, C_CH - 1],
                                              EM[K:2 * K, :, V_BI, C_CH - 1])
                    if s == 15:
                        # mirror bwd rows V_BI..15 (combine u 4..15) on the
                        # idle Scalar engine, overlapping remaining scan steps
                        nc.scalar.activation(
                            Sb0h[K:2 * K, :, POS - 16:16, :],
                            S2[K:2 * K, :, :, :][:, :, 15:POS - 17:-1, :],
                            ACTF.Copy)
                # mirror bwd rows 16..POS-1 (combine u 0..POS-16)
                nc.scalar.activation(
                    Sb0h[K:2 * K, :, 0:POS - 16, :],
                    S2[K:2 * K, :, :, :][:, :, POS - 1:15:-1, :],
                    ACTF.Copy)

                # V recompute + combine + transpose + rownorm + out per st:
                # V[u] = p_(t-1) @ eUn from stored fwd states (rows 7..22),
                # combined with mirrored bwd states straight from PSUM.
                eUn_l = blk[0:K, 0:K]
                for st in range(BL):
                    ps_v = ps_v_pool.tile([K, S_CH * C_CH], f32, tag="psv")
                    nc.tensor.matmul(ps_v[:], eUn_l,
                                     S2[0:K, st, V_BI - 1:V_BI + 15, :].opt(),
                                     start=True, stop=True)
                    # vc[k, r, u1, c] = V[u = 4*u1 + r, c] * w  (r-group
                    # contiguous so each transpose lhsT merges to 2D)
                    vc = opool.tile([K, 4, 4, C_CH], bf16, tag="vc")
                    nc.vector.tensor_tensor(
                        vc[:].rearrange("k r u1 c -> k u1 r c"),
                        ps_v[:].rearrange("k (u1 r c) -> k u1 r c", u1=4, r=4),
                        Sb0h[K:2 * K, st, :, :].rearrange(
                            "k (u1 r) c -> k u1 r c", r=4),
                        op=ALU.mult)
                    if st == 0:
                        # chunk 0, u 0 (t=0): v_0 = 1, so marginal = w_0
                        nc.vector.tensor_copy(vc[:, 0:1, 0:1, 0:1],
                                              Sb0h[K:2 * K, 0:1, 0:1, 0:1])
                    # real GEMM against [I | ones]: cols 0..32 = slab^T,
                    # col 32 = rowsums (free normalization denominator)
                    ps_o = ps_o_pool.tile([128, 4, K + 1], f32, tag="pso")
                    for r in range(4):
                        nc.tensor.matmul(ps_o[:, r, :], vc[:, r, :, :].opt(),
                                         idk[:], start=True, stop=True)
                    rc = opool.tile([128, 4], f32, tag="rc")
                    nc.vector.reciprocal(rc[:], ps_o[:, :, K])
                    o_sb = opool.tile([128, 4, K], f32, tag="osb")
                    nc.vector.tensor_tensor(o_sb[:], ps_o[:, :, 0:K],
                                            rc[:].to_broadcast((128, 4, K)),
                                            op=ALU.mult)
                    # partition p = u1*32 + c, free (r, k); row t = 16c + 4u1 + r
                    dst = o_h.ap()[st * 512:(st + 1) * 512, :].rearrange(
                        "(c u1 r) k -> u1 c r k", c=C_CH, u1=4)
                    eng = nc.sync if st % 2 == 0 else nc.scalar
                    eng.dma_start(dst, o_sb[:])
    if finalize:
        nc.finalize()
    return nc


_NC_CACHE = {}


def _get_nc():
    if "nc" not in _NC_CACHE:
        _NC_CACHE["nc"] = build_nc()
    return _NC_CACHE["nc"]


def _prep_x(x):
    """Per-core x^T bf16 shards: [D, ROWS] contiguous."""
    x = np.asarray(x, np.float32)
    shards = []
    for i in range(NCORES):
        xs = x[i * BL:(i + 1) * BL].reshape(ROWS, D).astype(ml_dtypes.bfloat16)
        shards.append(np.ascontiguousarray(xs.T))
    return shards


def _prep_consts(W, U, b):
    W = np.asarray(W, np.float32)
    U = np.asarray(U, np.float32)
    b = np.asarray(b, np.float32).reshape(K)
    wn = (W - W[:, 0:1]).astype(ml_dtypes.bfloat16)          # [D, K]
    wn = np.ascontiguousarray(
        wn.reshape(8, 128, K).transpose(1, 0, 2).reshape(128, 8 * K))
    eU = np.exp(U)
    blk = np.zeros((2 * K, 2 * K), np.float32)
    blk[0:K, 0:K] = eU
    blk[K:2 * K, K:2 * K] = eU.T
    bn = (b - b[0] + np.float32(LOG_CU)).reshape(K, 1).astype(np.float32)
    idk = np.zeros((K, K + 1), np.float32)
    idk[:, 0:K] = np.eye(K, dtype=np.float32)
    idk[:, K] = 1.0
    return wn, blk.astype(ml_dtypes.bfloat16), bn, idk.astype(ml_dtypes.bfloat16)


def kernel(x, W, U, b):
    from concourse.bass_utils import run_bass_kernel_spmd

    nc = _get_nc()
    xts = _prep_x(x)
    wn, blk, bn, idk = _prep_consts(W, U, b)
    in_maps = [
        {"x": xts[i], "wn": wn, "blk": blk, "bn": bn, "idk": idk}
        for i in range(NCORES)
    ]
    res = run_bass_kernel_spmd(nc, in_maps, list(range(NCORES)),
                               trace=os.environ.get("CRF_TRACE", "") == "1")
    out = np.concatenate(
        [res.results[i]["out"].reshape(BL, T, K) for i in range(NCORES)], axis=0)
    return out


if __name__ == "__main__":
    xs = np.random.randn(B, T, D).astype(np.float32)
    Ws = (np.random.randn(D, K) / np.sqrt(D)).astype(np.float32)
    Us = (np.random.randn(K, K) * 0.1).astype(np.float32)
    bs = np.zeros(K, np.float32)
    o = kernel(xs, Ws, Us, bs)
    print(o.shape, o.dtype, o[0, 0, :4])


# revision 23
# speedup vs baseline: 1.0232x; 1.0232x over previous
"""CRF forward-backward marginals on 8 Trainium2 NeuronCores.

Strategy (hardcoded for B=64, T=512, D=1024, K=32, 8 cores):
  - Data-parallel over batch: core i handles batches [8i, 8i+8).
  - Host-side prep: x^T bf16 per core (no on-chip transposes, half DMA);
    constants precomputed on host: wn = W - W[:,0] (bf16, chunk-major),
    bn = b - b[0] + log(1/(K*e)), blk = diag(exp(U), exp(U)^T),
    idk33 = [I_32 | ones] for fused transpose+rowsum.
  - Emissions: E'^T = exp(x @ wn + bn) via bf16 matmul, d-chunk-outer with
    8 concurrent PSUM groups so DMA overlaps the accumulating matmuls.
  - Fused scan: states for both directions live in one [64, ...] tile
    (fwd partitions 0:32, bwd 32:64, both step-indexed); one [64,64]
    block-diagonal matmul + one tensor_tensor per step:
      fwd:  p_s = (p_{s-1} @ eUn) * Em_f[s]
      bwd:  w_s = (w_{s-1} @ eUn^T) * Em_b[s]   (Em_b position-reversed)
    Time-parallelized over 32 chunks of 16 steps with 8 burn-in steps;
    chunk 0 fwd / chunk 31 bwd exactly re-init after burn-in.
    V (fwd pre-multiply) copied per step by Scalar; bwd states mirrored to
    a combine-ready base-0 tile by GpSimd.
  - Combine + PE transpose (with ones-column producing rowsums for free) +
    rownorm + 512B-contiguous DMA out.
"""

import os
import sys

import numpy as np
import ml_dtypes

sys.path.insert(0, "/opt/trn_rl_repo")

import concourse.bass as bass  # noqa: E402
import concourse.bacc as bacc  # noqa: E402
import concourse.mybir as mybir  # noqa: E402
from concourse import tile  # noqa: E402

B, T, D, K = 64, 512, 1024, 32
NCORES = 8
BL = B // NCORES            # 8 batches per core
ROWS = BL * T               # 4096 rows per core
S_CH = 16                   # chunk length
V_BI = 4                    # burn-in positions
C_CH = T // S_CH            # 32 chunks
CHAINS = BL * C_CH          # 256 parallel chains
POS = S_CH + V_BI           # 24 scan positions per direction
LOG_CU = -(np.log(K) + 1.0)  # log(1/(K*e)) folded into exp(U)

f32 = mybir.dt.float32
f32r = mybir.dt.float32r
bf16 = mybir.dt.bfloat16
AX = mybir.AxisListType
ALU = mybir.AluOpType
ACTF = mybir.ActivationFunctionType


def build_nc(finalize=True):
    nc = bacc.Bacc("TRN2", target_bir_lowering=False)
    x_h = nc.declare_dram_parameter("x", [D, ROWS], bf16, isOutput=False)
    wn_h = nc.declare_dram_parameter("wn", [128, 8 * K], bf16, isOutput=False)
    blk_h = nc.declare_dram_parameter("blk", [2 * K, 2 * K], bf16, isOutput=False)
    bn_h = nc.declare_dram_parameter("bn", [K, 1], f32, isOutput=False)
    idk_h = nc.declare_dram_parameter("idk", [K, K + 1], bf16, isOutput=False)
    o_h = nc.declare_dram_parameter("out", [ROWS, K], f32, isOutput=True)

    with tile.TileContext(nc) as tc:
        with (
            tc.tile_pool(name="const", bufs=1) as cpool,
            tc.tile_pool(name="stores", bufs=1) as spool,
        ):
            # constants on the scalar queue (tiny), x on the sync queue.
            wn3 = cpool.tile([128, 8, K], bf16)
            nc.scalar.dma_start(wn3[:], wn_h.ap().rearrange("p (n k) -> p n k", n=8))
            blk = cpool.tile([2 * K, 2 * K], bf16)
            nc.scalar.dma_start(blk[:], blk_h.ap())
            bn = cpool.tile([K, 1], f32)
            nc.scalar.dma_start(bn[:], bn_h.ap())
            idk = cpool.tile([K, K + 1], bf16)
            nc.scalar.dma_start(idk[:], idk_h.ap())

            # x: first d-chunk in 4 column pieces so the first matmuls can
            # start early; remaining chunks as whole 1MB tiles.
            xt = spool.tile([128, 8, ROWS], bf16)
            for p in range(2):
                nc.sync.dma_start(
                    xt[:, 0, p * 2048:(p + 1) * 2048],
                    x_h.ap()[0:128, p * 2048:(p + 1) * 2048])
            for db in range(1, 8):
                nc.sync.dma_start(xt[:, db, :],
                                  x_h.ap()[db * 128:(db + 1) * 128, :])

            # EM[p, b, s, c]: step-indexed emissions; fwd half (p<32) row s =
            # E'[16c + s - 8], bwd half row s = E'[16c + 23 - s]
            CU = float(np.exp(LOG_CU))
            EM = spool.tile([2 * K, BL, POS, C_CH], f32)
            S2 = spool.tile([2 * K, BL, POS, C_CH], bf16)
            Sb0h = spool.tile([2 * K, BL, S_CH, C_CH], bf16)
            nc.gpsimd.memset(EM[0:K, :, 0:V_BI, 0], CU)
            nc.gpsimd.memset(EM[K:2 * K, :, 0:V_BI, C_CH - 1], CU)

            warm = cpool.tile([128, 512], bf16)
            nc.gpsimd.memset(warm[:], 0.0)

            # ------------- emission -------------
            with tc.tile_pool(name="ps_e", bufs=1, space="PSUM") as ps_e_pool:
                e_ps = [ps_e_pool.tile([K, 512], f32, tag=f"e{st}", name=f"e_ps{st}")
                        for st in range(BL)]
                # dummy matmuls ramp the PE p-state while x streams in;
                # results land in e_ps[0] and are overwritten by the real
                # start=True matmul
                for w in range(4):
                    nc.tensor.matmul(e_ps[0][:], warm[:, 0:K], warm[:],
                                     start=True, stop=True)
                for db in range(8):
                    for st in range(BL):
                        nc.tensor.matmul(
                            e_ps[st][:], wn3[:, db, :],
                            xt[:, db, st * 512:(st + 1) * 512],
                            start=(db == 0), stop=(db == 7),
                        )
                        if db == 7:
                            eview = e_ps[st][:].rearrange("k (c u) -> k u c",
                                                          u=S_CH)
                            # fwd main rows [V_BI, V_BI+16)
                            nc.scalar.activation(
                                EM[0:K, st, V_BI:V_BI + 16, :], eview,
                                ACTF.Exp, bias=bn[:, 0:1])
                            # bwd main rows: row (V_BI+15-u) = fwd main row
                            # V_BI+u (position-reversed copy)
                            nc.vector.tensor_copy(
                                EM[K:2 * K, st, :, :][
                                    :, V_BI + 15:V_BI - 1:-1, :],
                                EM[0:K, st, V_BI:V_BI + 16, :])
                            # head dups (rows 0..V_BI): fwd from chunk c-1,
                            # bwd from chunk c+1 (+16 rule)
                            nc.vector.tensor_copy(
                                EM[0:K, st, 0:V_BI, 1:C_CH],
                                EM[0:K, st, 16:16 + V_BI, 0:C_CH - 1])
                            nc.gpsimd.tensor_copy(
                                EM[K:2 * K, st, 0:V_BI, 0:C_CH - 1],
                                EM[K:2 * K, st, 16:16 + V_BI, 1:C_CH])

            # ------------- fused scan + combine + output -------------
            with (
                tc.tile_pool(name="outsb", bufs=3) as opool,
                tc.tile_pool(name="ps_s", bufs=2, space="PSUM") as ps_s_pool,
                tc.tile_pool(name="ps_v", bufs=2, space="PSUM") as ps_v_pool,
                tc.tile_pool(name="ps_o", bufs=2, space="PSUM") as ps_o_pool,
            ):
                blk_r = blk[:]
                HB = BL // 2
                for s in range(POS):
                    if s == 0:
                        nc.vector.tensor_copy(S2[:, 0:HB, 0, :],
                                              EM[:, 0:HB, 0, :])
                        nc.vector.tensor_copy(S2[:, HB:BL, 0, :],
                                              EM[:, HB:BL, 0, :])
                        continue
                    # two independent half-batch chains so the MM of one half
                    # overlaps the TT of the other (hides serial latency)
                    for h in range(2):
                        bs = slice(h * HB, (h + 1) * HB)
                        ps2 = ps_s_pool.tile([2 * K, HB * C_CH], f32,
                                             tag=f"ps{h}")
                        nc.tensor.matmul(ps2[:], blk_r,
                                         S2[:, bs, s - 1, :].opt(),
                                         start=True, stop=True)
                        ps3 = ps2[:].rearrange("p (b c) -> p b c", b=HB)
                        nc.vector.tensor_tensor(S2[:, bs, s, :], ps3,
                                                EM[:, bs, s, :], op=ALU.mult)
                    if s == V_BI:
                        # exact re-inits once burn-in is done
                        nc.vector.tensor_copy(S2[0:K, :, V_BI, 0],
                                              EM[0:K, :, V_BI, 0])
                        nc.vector.tensor_copy(S2[K:2 * K, :, V_BI, C_CH - 1],
                                              EM[K:2 * K, :, V_BI, C_CH - 1])
                    if s == 15:
                        # mirror bwd rows V_BI..15 (combine u 4..15) on the
                        # idle Scalar engine, overlapping remaining scan steps
                        nc.scalar.activation(
                            Sb0h[K:2 * K, :, POS - 16:16, :],
                            S2[K:2 * K, :, :, :][:, :, 15:POS - 17:-1, :],
                            ACTF.Copy)
                # mirror bwd rows 16..POS-1 (combine u 0..POS-16)
                nc.scalar.activation(
                    Sb0h[K:2 * K, :, 0:POS - 16, :],
                    S2[K:2 * K, :, :, :][:, :, POS - 1:15:-1, :],
                    ACTF.Copy)

                # V recompute + combine + transpose + rownorm + out per st:
                # V[u] = p_(t-1) @ eUn from stored fwd states (rows 7..22),
                # combined with mirrored bwd states straight from PSUM.
                eUn_l = blk[0:K, 0:K]
                for st in range(BL):
                    ps_v = ps_v_pool.tile([K, S_CH * C_CH], f32, tag="psv")
                    nc.tensor.matmul(ps_v[:], eUn_l,
                                     S2[0:K, st, V_BI - 1:V_BI + 15, :].opt(),
                                     start=True, stop=True)
                    # vc[k, r, u1, c] = V[u = 4*u1 + r, c] * w  (r-group
                    # contiguous so each transpose lhsT merges to 2D)
                    vc = opool.tile([K, 4, 4, C_CH], bf16, tag="vc")
                    nc.vector.tensor_tensor(
                        vc[:].rearrange("k r u1 c -> k u1 r c"),
                        ps_v[:].rearrange("k (u1 r c) -> k u1 r c", u1=4, r=4),
                        Sb0h[K:2 * K, st, :, :].rearrange(
                            "k (u1 r) c -> k u1 r c", r=4),
                        op=ALU.mult)
                    if st == 0:
                        # chunk 0, u 0 (t=0): v_0 = 1, so marginal = w_0
                        nc.vector.tensor_copy(vc[:, 0:1, 0:1, 0:1],
                                              Sb0h[K:2 * K, 0:1, 0:1, 0:1])
                    # real GEMM against [I | ones]: cols 0..32 = slab^T,
                    # col 32 = rowsums (free normalization denominator)
                    ps_o = ps_o_pool.tile([128, 4, K + 1], f32, tag="pso")
                    for r in range(4):
                        nc.tensor.matmul(ps_o[:, r, :], vc[:, r, :, :].opt(),
                                         idk[:], start=True, stop=True)
                    rc = opool.tile([128, 4], f32, tag="rc")
                    nc.vector.reciprocal(rc[:], ps_o[:, :, K])
                    o_sb = opool.tile([128, 4, K], f32, tag="osb")
                    nc.vector.tensor_tensor(o_sb[:], ps_o[:, :, 0:K],
                                            rc[:].to_broadcast((128, 4, K)),
                                            op=ALU.mult)
                    # partition p = u1*32 + c, free (r, k); row t = 16c + 4u1 + r
                    dst = o_h.ap()[st * 512:(st + 1) * 512, :].rearrange(
                        "(c u1 r) k -> u1 c r k", c=C_CH, u1=4)
                    eng = nc.sync if st % 2 == 0 else nc.scalar
                    eng.dma_start(dst, o_sb[:])
    if finalize:
        nc.finalize()
    return nc


_NC_CACHE = {}


def _get_nc():
    if "nc" not in _NC_CACHE:
        _NC_CACHE["nc"] = build_nc()
    return _NC_CACHE["nc"]


def _prep_x(x):
    """Per-core x^T bf16 shards: [D, ROWS] contiguous."""
    x = np.asarray(x, np.float32)
    shards = []
    for i in range(NCORES):
        xs = x[i * BL:(i + 1) * BL].reshape(ROWS, D).astype(ml_dtypes.bfloat16)
        shards.append(np.ascontiguousarray(xs.T))
    return shards


def _prep_consts(W, U, b):
    W = np.asarray(W, np.float32)
    U = np.asarray(U, np.float32)
    b = np.asarray(b, np.float32).reshape(K)
    wn = (W - W[:, 0:1]).astype(ml_dtypes.bfloat16)          # [D, K]
    wn = np.ascontiguousarray(
        wn.reshape(8, 128, K).transpose(1, 0, 2).reshape(128, 8 * K))
    eU = np.exp(U)
    blk = np.zeros((2 * K, 2 * K), np.float32)
    blk[0:K, 0:K] = eU
    blk[K:2 * K, K:2 * K] = eU.T
    bn = (b - b[0] + np.float32(LOG_CU)).reshape(K, 1).astype(np.float32)
    idk = np.zeros((K, K + 1), np.float32)
    idk[:, 0:K] = np.eye(K, dtype=np.float32)
    idk[:, K] = 1.0
    return wn, blk.astype(ml_dtypes.bfloat16), bn, idk.astype(ml_dtypes.bfloat16)


def kernel(x, W, U, b):
    from concourse.bass_utils import run_bass_kernel_spmd

    nc = _get_nc()
    xts = _prep_x(x)
    wn, blk, bn, idk = _prep_consts(W, U, b)
    in_maps = [
        {"x": xts[i], "wn": wn, "blk": blk, "bn": bn, "idk": idk}
        for i in range(NCORES)
    ]
    res = run_bass_kernel_spmd(nc, in_maps, list(range(NCORES)),
                               trace=os.environ.get("CRF_TRACE", "") == "1")
    out = np.concatenate(
        [res.results[i]["out"].reshape(BL, T, K) for i in range(NCORES)], axis=0)
    return out


if __name__ == "__main__":
    xs = np.random.randn(B, T, D).astype(np.float32)
    Ws = (np.random.randn(D, K) / np.sqrt(D)).astype(np.float32)
    Us = (np.random.randn(K, K) * 0.1).astype(np.float32)
    bs = np.zeros(K, np.float32)
    o = kernel(xs, Ws, Us, bs)
    print(o.shape, o.dtype, o[0, 0, :4])


# revision 24
# speedup vs baseline: 1.0436x; 1.0199x over previous
"""CRF forward-backward marginals on 8 Trainium2 NeuronCores.

Strategy (hardcoded for B=64, T=512, D=1024, K=32, 8 cores):
  - Data-parallel over batch: core i handles batches [8i, 8i+8).
  - Host-side prep: x^T bf16 per core (no on-chip transposes, half DMA);
    constants precomputed on host: wn = W - W[:,0] (bf16, chunk-major),
    bn = b - b[0] + log(1/(K*e)), blk = diag(exp(U), exp(U)^T),
    idk33 = [I_32 | ones] for fused transpose+rowsum.
  - Emissions: E'^T = exp(x @ wn + bn) via bf16 matmul, d-chunk-outer with
    8 concurrent PSUM groups so DMA overlaps the accumulating matmuls.
  - Fused scan: states for both directions live in one [64, ...] tile
    (fwd partitions 0:32, bwd 32:64, both step-indexed); one [64,64]
    block-diagonal matmul + one tensor_tensor per step:
      fwd:  p_s = (p_{s-1} @ eUn) * Em_f[s]
      bwd:  w_s = (w_{s-1} @ eUn^T) * Em_b[s]   (Em_b position-reversed)
    Time-parallelized over 32 chunks of 16 steps with 8 burn-in steps;
    chunk 0 fwd / chunk 31 bwd exactly re-init after burn-in.
    V (fwd pre-multiply) copied per step by Scalar; bwd states mirrored to
    a combine-ready base-0 tile by GpSimd.
  - Combine + PE transpose (with ones-column producing rowsums for free) +
    rownorm + 512B-contiguous DMA out.
"""

import os
import sys

import numpy as np
import ml_dtypes

sys.path.insert(0, "/opt/trn_rl_repo")

import concourse.bass as bass  # noqa: E402
import concourse.bacc as bacc  # noqa: E402
import concourse.mybir as mybir  # noqa: E402
from concourse import tile  # noqa: E402

B, T, D, K = 64, 512, 1024, 32
NCORES = 8
BL = B // NCORES            # 8 batches per core
ROWS = BL * T               # 4096 rows per core
S_CH = 16                   # chunk length
V_BI = 4                    # burn-in positions
C_CH = T // S_CH            # 32 chunks
CHAINS = BL * C_CH          # 256 parallel chains
POS = S_CH + V_BI           # 24 scan positions per direction
LOG_CU = -(np.log(K) + 1.0)  # log(1/(K*e)) folded into exp(U)

f32 = mybir.dt.float32
f32r = mybir.dt.float32r
bf16 = mybir.dt.bfloat16
AX = mybir.AxisListType
ALU = mybir.AluOpType
ACTF = mybir.ActivationFunctionType


def build_nc(finalize=True):
    nc = bacc.Bacc("TRN2", target_bir_lowering=False)
    x_h = nc.declare_dram_parameter("x", [D, ROWS], bf16, isOutput=False)
    wn_h = nc.declare_dram_parameter("wn", [128, 8 * K], bf16, isOutput=False)
    blk_h = nc.declare_dram_parameter("blk", [2 * K, 2 * K], bf16, isOutput=False)
    bn_h = nc.declare_dram_parameter("bn", [K, 1], f32, isOutput=False)
    idk_h = nc.declare_dram_parameter("idk", [K, K + 1], bf16, isOutput=False)
    o_h = nc.declare_dram_parameter("out", [ROWS, K], f32, isOutput=True)

    with tile.TileContext(nc) as tc:
        with (
            tc.tile_pool(name="const", bufs=1) as cpool,
            tc.tile_pool(name="stores", bufs=1) as spool,
        ):
            # constants on the scalar queue (tiny), x on the sync queue.
            wn3 = cpool.tile([128, 8, K], bf16)
            nc.scalar.dma_start(wn3[:], wn_h.ap().rearrange("p (n k) -> p n k", n=8))
            blk = cpool.tile([2 * K, 2 * K], bf16)
            nc.scalar.dma_start(blk[:], blk_h.ap())
            bn = cpool.tile([K, 1], f32)
            nc.scalar.dma_start(bn[:], bn_h.ap())
            idk = cpool.tile([K, K + 1], bf16)
            nc.scalar.dma_start(idk[:], idk_h.ap())

            # x: first d-chunk in 4 column pieces so the first matmuls can
            # start early; remaining chunks as whole 1MB tiles.
            xt = spool.tile([128, 8, ROWS], bf16)
            for p in range(2):
                nc.sync.dma_start(
                    xt[:, 0, p * 2048:(p + 1) * 2048],
                    x_h.ap()[0:128, p * 2048:(p + 1) * 2048])
            for db in range(1, 8):
                nc.sync.dma_start(xt[:, db, :],
                                  x_h.ap()[db * 128:(db + 1) * 128, :])

            # EM[p, b, s, c]: step-indexed emissions; fwd half (p<32) row s =
            # E'[16c + s - 8], bwd half row s = E'[16c + 23 - s]
            CU = float(np.exp(LOG_CU))
            EM = spool.tile([2 * K, BL, POS, C_CH], f32)
            S2 = spool.tile([2 * K, BL, POS, C_CH], bf16)
            Sb0h = spool.tile([2 * K, BL, S_CH, C_CH], bf16)
            nc.gpsimd.memset(EM[0:K, :, 0:V_BI, 0], CU)
            nc.gpsimd.memset(EM[K:2 * K, :, 0:V_BI, C_CH - 1], CU)

            warm = cpool.tile([128, 512], bf16)
            nc.gpsimd.memset(warm[:], 0.0)

            # ------------- emission -------------
            with tc.tile_pool(name="ps_e", bufs=1, space="PSUM") as ps_e_pool:
                e_ps = [ps_e_pool.tile([K, 512], f32, tag=f"e{st}", name=f"e_ps{st}")
                        for st in range(BL)]
                # dummy matmuls ramp the PE p-state while x streams in;
                # results land in e_ps[0] and are overwritten by the real
                # start=True matmul
                for w in range(4):
                    nc.tensor.matmul(e_ps[0][:], warm[:, 0:K], warm[:],
                                     start=True, stop=True)
                for db in range(8):
                    for st in range(BL):
                        nc.tensor.matmul(
                            e_ps[st][:], wn3[:, db, :],
                            xt[:, db, st * 512:(st + 1) * 512],
                            start=(db == 0), stop=(db == 7),
                        )
                        if db == 7:
                            eview = e_ps[st][:].rearrange("k (c u) -> k u c",
                                                          u=S_CH)
                            # fwd main rows [V_BI, V_BI+16)
                            nc.scalar.activation(
                                EM[0:K, st, V_BI:V_BI + 16, :], eview,
                                ACTF.Exp, bias=bn[:, 0:1])
                            # bwd main rows: row (V_BI+15-u) = fwd main row
                            # V_BI+u (position-reversed copy)
                            nc.vector.tensor_copy(
                                EM[K:2 * K, st, :, :][
                                    :, V_BI + 15:V_BI - 1:-1, :],
                                EM[0:K, st, V_BI:V_BI + 16, :])
                            # head dups (rows 0..V_BI): fwd from chunk c-1,
                            # bwd from chunk c+1 (+16 rule)
                            nc.vector.tensor_copy(
                                EM[0:K, st, 0:V_BI, 1:C_CH],
                                EM[0:K, st, 16:16 + V_BI, 0:C_CH - 1])
                            nc.vector.tensor_copy(
                                EM[K:2 * K, st, 0:V_BI, 0:C_CH - 1],
                                EM[K:2 * K, st, 16:16 + V_BI, 1:C_CH])

            # ------------- fused scan + combine + output -------------
            with (
                tc.tile_pool(name="outsb", bufs=3) as opool,
                tc.tile_pool(name="ps_s", bufs=2, space="PSUM") as ps_s_pool,
                tc.tile_pool(name="ps_v", bufs=2, space="PSUM") as ps_v_pool,
                tc.tile_pool(name="ps_o", bufs=2, space="PSUM") as ps_o_pool,
            ):
                blk_r = blk[:]
                HB = BL // 2
                for s in range(POS):
                    if s == 0:
                        nc.vector.tensor_copy(S2[:, 0:HB, 0, :],
                                              EM[:, 0:HB, 0, :])
                        nc.vector.tensor_copy(S2[:, HB:BL, 0, :],
                                              EM[:, HB:BL, 0, :])
                        continue
                    # two independent half-batch chains so the MM of one half
                    # overlaps the TT of the other (hides serial latency)
                    for h in range(2):
                        bs = slice(h * HB, (h + 1) * HB)
                        ps2 = ps_s_pool.tile([2 * K, HB * C_CH], f32,
                                             tag=f"ps{h}")
                        nc.tensor.matmul(ps2[:], blk_r,
                                         S2[:, bs, s - 1, :].opt(),
                                         start=True, stop=True)
                        ps3 = ps2[:].rearrange("p (b c) -> p b c", b=HB)
                        nc.vector.tensor_tensor(S2[:, bs, s, :], ps3,
                                                EM[:, bs, s, :], op=ALU.mult)
                    if s == V_BI:
                        # exact re-inits once burn-in is done
                        nc.vector.tensor_copy(S2[0:K, :, V_BI, 0],
                                              EM[0:K, :, V_BI, 0])
                        nc.vector.tensor_copy(S2[K:2 * K, :, V_BI, C_CH - 1],
                                              EM[K:2 * K, :, V_BI, C_CH - 1])
                    if s == 15:
                        # mirror bwd rows V_BI..15 (combine u 4..15) on the
                        # idle Scalar engine, overlapping remaining scan steps
                        nc.scalar.activation(
                            Sb0h[K:2 * K, :, POS - 16:16, :],
                            S2[K:2 * K, :, :, :][:, :, 15:POS - 17:-1, :],
                            ACTF.Copy)
                # mirror bwd rows 16..POS-1 (combine u 0..POS-16)
                nc.scalar.activation(
                    Sb0h[K:2 * K, :, 0:POS - 16, :],
                    S2[K:2 * K, :, :, :][:, :, POS - 1:15:-1, :],
                    ACTF.Copy)

                # V recompute + combine + transpose + rownorm + out per st:
                # V[u] = p_(t-1) @ eUn from stored fwd states (rows 7..22),
                # combined with mirrored bwd states straight from PSUM.
                eUn_l = blk[0:K, 0:K]
                for st in range(BL):
                    ps_v = ps_v_pool.tile([K, S_CH * C_CH], f32, tag="psv")
                    nc.tensor.matmul(ps_v[:], eUn_l,
                                     S2[0:K, st, V_BI - 1:V_BI + 15, :].opt(),
                                     start=True, stop=True)
                    # vc[k, r, u1, c] = V[u = 4*u1 + r, c] * w  (r-group
                    # contiguous so each transpose lhsT merges to 2D)
                    vc = opool.tile([K, 4, 4, C_CH], bf16, tag="vc")
                    nc.vector.tensor_tensor(
                        vc[:].rearrange("k r u1 c -> k u1 r c"),
                        ps_v[:].rearrange("k (u1 r c) -> k u1 r c", u1=4, r=4),
                        Sb0h[K:2 * K, st, :, :].rearrange(
                            "k (u1 r) c -> k u1 r c", r=4),
                        op=ALU.mult)
                    if st == 0:
                        # chunk 0, u 0 (t=0): v_0 = 1, so marginal = w_0
                        nc.vector.tensor_copy(vc[:, 0:1, 0:1, 0:1],
                                              Sb0h[K:2 * K, 0:1, 0:1, 0:1])
                    # real GEMM against [I | ones]: cols 0..32 = slab^T,
                    # col 32 = rowsums (free normalization denominator)
                    ps_o = ps_o_pool.tile([128, 4, K + 1], f32, tag="pso")
                    for r in range(4):
                        nc.tensor.matmul(ps_o[:, r, :], vc[:, r, :, :].opt(),
                                         idk[:], start=True, stop=True)
                    rc = opool.tile([128, 4], f32, tag="rc")
                    nc.vector.reciprocal(rc[:], ps_o[:, :, K])
                    o_sb = opool.tile([128, 4, K], f32, tag="osb")
                    nc.vector.tensor_tensor(o_sb[:], ps_o[:, :, 0:K],
                                            rc[:].to_broadcast((128, 4, K)),
                                            op=ALU.mult)
                    # partition p = u1*32 + c, free (r, k); row t = 16c + 4u1 + r
                    dst = o_h.ap()[st * 512:(st + 1) * 512, :].rearrange(
                        "(c u1 r) k -> u1 c r k", c=C_CH, u1=4)
                    eng = nc.sync if st % 2 == 0 else nc.scalar
                    eng.dma_start(dst, o_sb[:])
    if finalize:
        nc.finalize()
    return nc


_NC_CACHE = {}


def _get_nc():
    if "nc" not in _NC_CACHE:
        _NC_CACHE["nc"] = build_nc()
    return _NC_CACHE["nc"]


def _prep_x(x):
    """Per-core x^T bf16 shards: [D, ROWS] contiguous."""
    x = np.asarray(x, np.float32)
    shards = []
    for i in range(NCORES):
        xs = x[i * BL:(i + 1) * BL].reshape(ROWS, D).astype(ml_dtypes.bfloat16)
        shards.append(np.ascontiguousarray(xs.T))
    return shards


def _prep_consts(W, U, b):
    W = np.asarray(W, np.float32)
    U = np.asarray(U, np.float32)
    b = np.asarray(b, np.float32).reshape(K)
    wn = (W - W[:, 0:1]).astype(ml_dtypes.bfloat16)          # [D, K]
    wn = np.ascontiguousarray(
        wn.reshape(8, 128, K).transpose(1, 0, 2).reshape(128, 8 * K))
    eU = np.exp(U)
    blk = np.zeros((2 * K, 2 * K), np.float32)
    blk[0:K, 0:K] = eU
    blk[K:2 * K, K:2 * K] = eU.T
    bn = (b - b[0] + np.float32(LOG_CU)).reshape(K, 1).astype(np.float32)
    idk = np.zeros((K, K + 1), np.float32)
    idk[:, 0:K] = np.eye(K, dtype=np.float32)
    idk[:, K] = 1.0
    return wn, blk.astype(ml_dtypes.bfloat16), bn, idk.astype(ml_dtypes.bfloat16)


def kernel(x, W, U, b):
    from concourse.bass_utils import run_bass_kernel_spmd

    nc = _get_nc()
    xts = _prep_x(x)
    wn, blk, bn, idk = _prep_consts(W, U, b)
    in_maps = [
        {"x": xts[i], "wn": wn, "blk": blk, "bn": bn, "idk": idk}
        for i in range(NCORES)
    ]
    res = run_bass_kernel_spmd(nc, in_maps, list(range(NCORES)),
                               trace=os.environ.get("CRF_TRACE", "") == "1")
    out = np.concatenate(
        [res.results[i]["out"].reshape(BL, T, K) for i in range(NCORES)], axis=0)
    return out


if __name__ == "__main__":
    xs = np.random.randn(B, T, D).astype(np.float32)
    Ws = (np.random.randn(D, K) / np.sqrt(D)).astype(np.float32)
    Us = (np.random.randn(K, K) * 0.1).astype(np.float32)
    bs = np.zeros(K, np.float32)
    o = kernel(xs, Ws, Us, bs)
    print(o.shape, o.dtype, o[0, 0, :4])


# revision 25
# speedup vs baseline: 1.0573x; 1.0131x over previous
"""CRF forward-backward marginals on 8 Trainium2 NeuronCores.

Strategy (hardcoded for B=64, T=512, D=1024, K=32, 8 cores):
  - Data-parallel over batch: core i handles batches [8i, 8i+8).
  - Host-side prep: x^T bf16 per core (no on-chip transposes, half DMA);
    constants precomputed on host: wn = W - W[:,0] (bf16, chunk-major),
    bn = b - b[0] + log(1/(K*e)), blk = diag(exp(U), exp(U)^T),
    idk33 = [I_32 | ones] for fused transpose+rowsum.
  - Emissions: E'^T = exp(x @ wn + bn) via bf16 matmul, d-chunk-outer with
    8 concurrent PSUM groups so DMA overlaps the accumulating matmuls.
  - Fused scan: states for both directions live in one [64, ...] tile
    (fwd partitions 0:32, bwd 32:64, both step-indexed); one [64,64]
    block-diagonal matmul + one tensor_tensor per step:
      fwd:  p_s = (p_{s-1} @ eUn) * Em_f[s]
      bwd:  w_s = (w_{s-1} @ eUn^T) * Em_b[s]   (Em_b position-reversed)
    Time-parallelized over 32 chunks of 16 steps with 8 burn-in steps;
    chunk 0 fwd / chunk 31 bwd exactly re-init after burn-in.
    V (fwd pre-multiply) copied per step by Scalar; bwd states mirrored to
    a combine-ready base-0 tile by GpSimd.
  - Combine + PE transpose (with ones-column producing rowsums for free) +
    rownorm + 512B-contiguous DMA out.
"""

import os
import sys

import numpy as np
import ml_dtypes

sys.path.insert(0, "/opt/trn_rl_repo")

import concourse.bass as bass  # noqa: E402
import concourse.bacc as bacc  # noqa: E402
import concourse.mybir as mybir  # noqa: E402
from concourse import tile  # noqa: E402

B, T, D, K = 64, 512, 1024, 32
NCORES = 8
BL = B // NCORES            # 8 batches per core
ROWS = BL * T               # 4096 rows per core
S_CH = 16                   # chunk length
V_BI = 4                    # burn-in positions
C_CH = T // S_CH            # 32 chunks
CHAINS = BL * C_CH          # 256 parallel chains
POS = S_CH + V_BI           # 24 scan positions per direction
LOG_CU = -(np.log(K) + 1.0)  # log(1/(K*e)) folded into exp(U)

f32 = mybir.dt.float32
f32r = mybir.dt.float32r
bf16 = mybir.dt.bfloat16
AX = mybir.AxisListType
ALU = mybir.AluOpType
ACTF = mybir.ActivationFunctionType


def build_nc(finalize=True):
    nc = bacc.Bacc("TRN2", target_bir_lowering=False)
    x_h = nc.declare_dram_parameter("x", [D, ROWS], bf16, isOutput=False)
    wn_h = nc.declare_dram_parameter("wn", [128, 8 * K], bf16, isOutput=False)
    blk_h = nc.declare_dram_parameter("blk", [2 * K, 2 * K], bf16, isOutput=False)
    bn_h = nc.declare_dram_parameter("bn", [K, 1], f32, isOutput=False)
    idk_h = nc.declare_dram_parameter("idk", [K, K + 1], bf16, isOutput=False)
    o_h = nc.declare_dram_parameter("out", [ROWS, K], f32, isOutput=True)

    with tile.TileContext(nc) as tc:
        with (
            tc.tile_pool(name="const", bufs=1) as cpool,
            tc.tile_pool(name="stores", bufs=1) as spool,
        ):
            # constants on the scalar queue (tiny), x on the sync queue.
            wn3 = cpool.tile([128, 8, K], bf16)
            nc.scalar.dma_start(wn3[:], wn_h.ap().rearrange("p (n k) -> p n k", n=8))
            blk = cpool.tile([2 * K, 2 * K], bf16)
            nc.scalar.dma_start(blk[:], blk_h.ap())
            bn = cpool.tile([K, 1], f32)
            nc.scalar.dma_start(bn[:], bn_h.ap())
            idk = cpool.tile([K, K + 1], bf16)
            nc.scalar.dma_start(idk[:], idk_h.ap())

            # x: first d-chunk in 4 column pieces so the first matmuls can
            # start early; remaining chunks as whole 1MB tiles.
            xt = spool.tile([128, 8, ROWS], bf16)
            for p in range(4):
                nc.sync.dma_start(
                    xt[:, 0, p * 1024:(p + 1) * 1024],
                    x_h.ap()[0:128, p * 1024:(p + 1) * 1024])
            for db in range(1, 8):
                nc.sync.dma_start(xt[:, db, :],
                                  x_h.ap()[db * 128:(db + 1) * 128, :])

            # EM[p, b, s, c]: step-indexed emissions; fwd half (p<32) row s =
            # E'[16c + s - 8], bwd half row s = E'[16c + 23 - s]
            CU = float(np.exp(LOG_CU))
            EM = spool.tile([2 * K, BL, POS, C_CH], f32)
            S2 = spool.tile([2 * K, BL, POS, C_CH], bf16)
            Sb0h = spool.tile([2 * K, BL, S_CH, C_CH], bf16)
            nc.gpsimd.memset(EM[0:K, :, 0:V_BI, 0], CU)
            nc.gpsimd.memset(EM[K:2 * K, :, 0:V_BI, C_CH - 1], CU)

            warm = cpool.tile([128, 512], bf16)
            nc.gpsimd.memset(warm[:], 0.0)

            # ------------- emission -------------
            with tc.tile_pool(name="ps_e", bufs=1, space="PSUM") as ps_e_pool:
                e_ps = [ps_e_pool.tile([K, 512], f32, tag=f"e{st}", name=f"e_ps{st}")
                        for st in range(BL)]
                # dummy matmuls ramp the PE p-state while x streams in;
                # results land in e_ps[0] and are overwritten by the real
                # start=True matmul
                for w in range(6):
                    nc.tensor.matmul(e_ps[0][:], warm[:, 0:K], warm[:],
                                     start=True, stop=True)
                for db in range(8):
                    for st in range(BL):
                        nc.tensor.matmul(
                            e_ps[st][:], wn3[:, db, :],
                            xt[:, db, st * 512:(st + 1) * 512],
                            start=(db == 0), stop=(db == 7),
                        )
                        if db == 7:
                            eview = e_ps[st][:].rearrange("k (c u) -> k u c",
                                                          u=S_CH)
                            # fwd main rows [V_BI, V_BI+16)
                            nc.scalar.activation(
                                EM[0:K, st, V_BI:V_BI + 16, :], eview,
                                ACTF.Exp, bias=bn[:, 0:1])
                            # bwd main rows: row (V_BI+15-u) = fwd main row
                            # V_BI+u (position-reversed copy)
                            nc.vector.tensor_copy(
                                EM[K:2 * K, st, :, :][
                                    :, V_BI + 15:V_BI - 1:-1, :],
                                EM[0:K, st, V_BI:V_BI + 16, :])
                            # head dups (rows 0..V_BI): fwd from chunk c-1,
                            # bwd from chunk c+1 (+16 rule)
                            nc.vector.tensor_copy(
                                EM[0:K, st, 0:V_BI, 1:C_CH],
                                EM[0:K, st, 16:16 + V_BI, 0:C_CH - 1])
                            nc.vector.tensor_copy(
                                EM[K:2 * K, st, 0:V_BI, 0:C_CH - 1],
                                EM[K:2 * K, st, 16:16 + V_BI, 1:C_CH])

            # ------------- fused scan + combine + output -------------
            with (
                tc.tile_pool(name="outsb", bufs=3) as opool,
                tc.tile_pool(name="ps_s", bufs=2, space="PSUM") as ps_s_pool,
                tc.tile_pool(name="ps_v", bufs=2, space="PSUM") as ps_v_pool,
                tc.tile_pool(name="ps_o", bufs=2, space="PSUM") as ps_o_pool,
            ):
                blk_r = blk[:]
                HB = BL // 2
                for s in range(POS):
                    if s == 0:
                        nc.vector.tensor_copy(S2[:, :, 0, :], EM[:, :, 0, :])
                        continue
                    # two independent half-batch chains so the MM of one half
                    # overlaps the TT of the other (hides serial latency)
                    for h in range(2):
                        bs = slice(h * HB, (h + 1) * HB)
                        ps2 = ps_s_pool.tile([2 * K, HB * C_CH], f32,
                                             tag=f"ps{h}")
                        nc.tensor.matmul(ps2[:], blk_r,
                                         S2[:, bs, s - 1, :].opt(),
                                         start=True, stop=True)
                        ps3 = ps2[:].rearrange("p (b c) -> p b c", b=HB)
                        nc.vector.tensor_tensor(S2[:, bs, s, :], ps3,
                                                EM[:, bs, s, :], op=ALU.mult)
                    if s == V_BI:
                        # exact re-inits once burn-in is done
                        nc.vector.tensor_copy(S2[0:K, :, V_BI, 0],
                                              EM[0:K, :, V_BI, 0])
                        nc.vector.tensor_copy(S2[K:2 * K, :, V_BI, C_CH - 1],
                                              EM[K:2 * K, :, V_BI, C_CH - 1])
                    if s == 15:
                        # mirror bwd rows V_BI..15 (combine u 4..15) on the
                        # idle Scalar engine, overlapping remaining scan steps
                        nc.scalar.activation(
                            Sb0h[K:2 * K, :, POS - 16:16, :],
                            S2[K:2 * K, :, :, :][:, :, 15:POS - 17:-1, :],
                            ACTF.Copy)
                # mirror bwd rows 16..POS-1 (combine u 0..POS-16)
                nc.scalar.activation(
                    Sb0h[K:2 * K, :, 0:POS - 16, :],
                    S2[K:2 * K, :, :, :][:, :, POS - 1:15:-1, :],
                    ACTF.Copy)

                # V recompute + combine + transpose + rownorm + out per st:
                # V[u] = p_(t-1) @ eUn from stored fwd states (rows 7..22),
                # combined with mirrored bwd states straight from PSUM.
                eUn_l = blk[0:K, 0:K]
                for st in range(BL):
                    ps_v = ps_v_pool.tile([K, S_CH * C_CH], f32, tag="psv")
                    nc.tensor.matmul(ps_v[:], eUn_l,
                                     S2[0:K, st, V_BI - 1:V_BI + 15, :].opt(),
                                     start=True, stop=True)
                    # vc[k, r, u1, c] = V[u = 4*u1 + r, c] * w  (r-group
                    # contiguous so each transpose lhsT merges to 2D)
                    vc = opool.tile([K, 4, 4, C_CH], bf16, tag="vc")
                    nc.vector.tensor_tensor(
                        vc[:].rearrange("k r u1 c -> k u1 r c"),
                        ps_v[:].rearrange("k (u1 r c) -> k u1 r c", u1=4, r=4),
                        Sb0h[K:2 * K, st, :, :].rearrange(
                            "k (u1 r) c -> k u1 r c", r=4),
                        op=ALU.mult)
                    if st == 0:
                        # chunk 0, u 0 (t=0): v_0 = 1, so marginal = w_0
                        nc.vector.tensor_copy(vc[:, 0:1, 0:1, 0:1],
                                              Sb0h[K:2 * K, 0:1, 0:1, 0:1])
                    # real GEMM against [I | ones]: cols 0..32 = slab^T,
                    # col 32 = rowsums (free normalization denominator)
                    ps_o = ps_o_pool.tile([128, 4, K + 1], f32, tag="pso")
                    for r in range(4):
                        nc.tensor.matmul(ps_o[:, r, :], vc[:, r, :, :].opt(),
                                         idk[:], start=True, stop=True)
                    rc = opool.tile([128, 4], f32, tag="rc")
                    nc.vector.reciprocal(rc[:], ps_o[:, :, K])
                    o_sb = opool.tile([128, 4, K], f32, tag="osb")
                    nc.vector.tensor_tensor(o_sb[:], ps_o[:, :, 0:K],
                                            rc[:].to_broadcast((128, 4, K)),
                                            op=ALU.mult)
                    # partition p = u1*32 + c, free (r, k); row t = 16c + 4u1 + r
                    dst = o_h.ap()[st * 512:(st + 1) * 512, :].rearrange(
                        "(c u1 r) k -> u1 c r k", c=C_CH, u1=4)
                    eng = nc.sync if st % 2 == 0 else nc.scalar
                    eng.dma_start(dst, o_sb[:])
    if finalize:
        nc.finalize()
    return nc


_NC_CACHE = {}


def _get_nc():
    if "nc" not in _NC_CACHE:
        _NC_CACHE["nc"] = build_nc()
    return _NC_CACHE["nc"]


def _prep_x(x):
    """Per-core x^T bf16 shards: [D, ROWS] contiguous."""
    x = np.asarray(x, np.float32)
    shards = []
    for i in range(NCORES):
        xs = x[i * BL:(i + 1) * BL].reshape(ROWS, D).astype(ml_dtypes.bfloat16)
        shards.append(np.ascontiguousarray(xs.T))
    return shards


def _prep_consts(W, U, b):
    W = np.asarray(W, np.float32)
    U = np.asarray(U, np.float32)
    b = np.asarray(b, np.float32).reshape(K)
    wn = (W - W[:, 0:1]).astype(ml_dtypes.bfloat16)          # [D, K]
    wn = np.ascontiguousarray(
        wn.reshape(8, 128, K).transpose(1, 0, 2).reshape(128, 8 * K))
    eU = np.exp(U)
    blk = np.zeros((2 * K, 2 * K), np.float32)
    blk[0:K, 0:K] = eU
    blk[K:2 * K, K:2 * K] = eU.T
    bn = (b - b[0] + np.float32(LOG_CU)).reshape(K, 1).astype(np.float32)
    idk = np.zeros((K, K + 1), np.float32)
    idk[:, 0:K] = np.eye(K, dtype=np.float32)
    idk[:, K] = 1.0
    return wn, blk.astype(ml_dtypes.bfloat16), bn, idk.astype(ml_dtypes.bfloat16)


def kernel(x, W, U, b):
    from concourse.bass_utils import run_bass_kernel_spmd

    nc = _get_nc()
    xts = _prep_x(x)
    wn, blk, bn, idk = _prep_consts(W, U, b)
    in_maps = [
        {"x": xts[i], "wn": wn, "blk": blk, "bn": bn, "idk": idk}
        for i in range(NCORES)
    ]
    res = run_bass_kernel_spmd(nc, in_maps, list(range(NCORES)),
                               trace=os.environ.get("CRF_TRACE", "") == "1")
    out = np.concatenate(
        [res.results[i]["out"].reshape(BL, T, K) for i in range(NCORES)], axis=0)
    return out


if __name__ == "__main__":
    xs = np.random.randn(B, T, D).astype(np.float32)
    Ws = (np.random.randn(D, K) / np.sqrt(D)).astype(np.float32)
    Us = (np.random.randn(K, K) * 0.1).astype(np.float32)
    bs = np.zeros(K, np.float32)
    o = kernel(xs, Ws, Us, bs)
    print(o.shape, o.dtype, o[0, 0, :4])
